# revision 1
# baseline (speedup 1.0000x reference)
"""GCN layer (out = D^-1/2 (A+I) D^-1/2 (x W^T + b)) on 8 trn2 NeuronCores.

Strategy:
  - Host: append self-loops, sort edges by dst, partition dst-blocks (128
    nodes) across cores (LPT-balanced), build a core-invariant static
    schedule (uniform SPMD program) with per-core data tables.
  - Device P1 (replicated): T[v] = rsqrt(deg[v]) * (x[v] @ W^T + b), written
    to DRAM in 4 subtables (int16-indexable), bf16.
  - Device P2: per dst-block: batched dma_gather of T rows per edge chunk
    (128 edges), selection matrix via is_equal (dst_rel vs iota), PE matmul
    S^T @ msg accumulating into PSUM, drain with rsqrt(deg_dst) scale.
"""

import math
import time
from contextlib import ExitStack

import ml_dtypes
import numpy as np

import concourse.bass as bass
import concourse.tile as tile
from concourse import bacc, mybir
from concourse.bass_utils import run_bass_kernel_spmd

F32 = mybir.dt.float32
BF16 = mybir.dt.bfloat16
I16 = mybir.dt.int16

# ---------------------------------------------------------------------------
# Host-side planning
# ---------------------------------------------------------------------------


class Plan:
    pass


def build_plan(src_all, dst_all, n_nodes, n_cores, d=128,
               subt_cap=32768, gb=32, sg=8, align_rows=1024):
    """src_all/dst_all: int64/int32 edge endpoints INCLUDING self loops.
    Returns Plan with static schedule + per-core data arrays."""
    t0 = time.time()
    p = Plan()
    p.d = d
    p.gb = gb          # chunks per gather batch
    p.sg = sg          # chunks per is_equal batch
    p.n_nodes = n_nodes
    p.n_cores = n_cores

    n_blocks_real = math.ceil(n_nodes / 128)
    slots = math.ceil(n_blocks_real / n_cores)
    p.slots = slots
    n_blocks = slots * n_cores
    p.n_pad = n_blocks * 128

    # subtable boundaries (each multiple of 128 rows, capacity <= subt_cap)
    n_subt = math.ceil(p.n_pad / subt_cap)
    base = (p.n_pad // n_subt) // align_rows * align_rows
    sizes = [base] * n_subt
    sizes[-1] = p.n_pad - base * (n_subt - 1)
    assert all(0 < s <= subt_cap for s in sizes), sizes
    p.subt_sizes = sizes
    p.subt_off = np.cumsum([0] + sizes)  # [n_subt+1]
    p.n_subt = n_subt
    p.align_rows = align_rows

    # sort edges by dst
    order = np.argsort(dst_all, kind="stable")
    dst_s = np.asarray(dst_all)[order].astype(np.int64)
    src_s = np.asarray(src_all)[order].astype(np.int64)
    rowptr = np.searchsorted(dst_s, np.arange(p.n_pad + 1))
    p.rowptr = rowptr

    blk_ptr = rowptr[::128]  # [n_blocks+1] edges per block boundaries
    blk_cnt = blk_ptr[1:] - blk_ptr[:-1]

    # per (block, subtable) edge counts -> chunks
    subt_of_src = np.searchsorted(p.subt_off[1:], src_s, side="right")
    # within each block, edges sorted by (subt, original order)
    blk_chunks = np.zeros((n_blocks, n_subt), dtype=np.int64)
    blk_edges = []  # per block: list per subt of (src_local, dst) arrays
    for b in range(n_blocks):
        lo, hi = blk_ptr[b], blk_ptr[b + 1]
        qs = subt_of_src[lo:hi]
        per_q = []
        for q in range(n_subt):
            m = qs == q
            sl = src_s[lo:hi][m] - p.subt_off[q]
            dl = dst_s[lo:hi][m] - b * 128
            per_q.append((sl, dl))
            blk_chunks[b, q] = math.ceil(len(sl) / 128) if len(sl) else 0
        blk_edges.append(per_q)

    # LPT assignment of blocks to cores by total chunk count
    tot = blk_chunks.sum(axis=1)
    order_blocks = np.argsort(-tot, kind="stable")
    core_loads = [0] * n_cores
    core_blocks = [[] for _ in range(n_cores)]
    for b in order_blocks:
        c = int(np.argmin(core_loads))
        core_loads[c] += int(tot[b])
        core_blocks[c].append(int(b))
    # per core: sort desc by total chunks (LPT order already desc-ish)
    for c in range(n_cores):
        core_blocks[c].sort(key=lambda b: -int(tot[b]))
        while len(core_blocks[c]) < slots:
            core_blocks[c].append(-1)  # ghost slot
    p.core_blocks = core_blocks  # [n_cores][slots] block id or -1

    # static schedule: k[s][q] = max over cores of chunks for that slot/subt
    k_sq = np.zeros((slots, n_subt), dtype=np.int64)
    for c in range(n_cores):
        for s in range(slots):
            b = core_blocks[c][s]
            if b >= 0:
                k_sq[s] = np.maximum(k_sq[s], blk_chunks[b])
    for s in range(slots):
        if k_sq[s].sum() == 0:
            k_sq[s, 0] = 1  # ensure psum gets start/stop
    p.k_sq = k_sq
    p.k_s = k_sq.sum(axis=1)  # chunks per slot
    n_chunks = int(p.k_s.sum())

    # chunk order (slot-major, subt groups in order) + per-subt stream pos
    chunk_q = np.zeros(n_chunks, dtype=np.int64)   # subtable of chunk
    chunk_slot = np.zeros(n_chunks, dtype=np.int64)
    chunk_pos = np.zeros(n_chunks, dtype=np.int64)  # stream position in its q
    qcount = [0] * n_subt
    ci = 0
    for s in range(slots):
        for q in range(n_subt):
            for _ in range(int(k_sq[s, q])):
                chunk_q[ci] = q
                chunk_slot[ci] = s
                chunk_pos[ci] = qcount[q]
                qcount[q] += 1
                ci += 1
    assert ci == n_chunks
    p.n_chunks = n_chunks
    p.chunk_q = chunk_q
    p.chunk_slot = chunk_slot
    p.chunk_pos = chunk_pos
    p.stream_len = [math.ceil(qcount[q] / gb) * gb if qcount[q] else 0
                    for q in range(n_subt)]   # in chunks, gb-aligned
    p.n_batches = [sl // gb for sl in p.stream_len]

    # per-core data tables
    p.core_idx = []      # per core: list per q of int16 [128, stream_len*128/16]
    p.core_dst_rel = []  # per core: f32->bf16 [128, n_chunks]
    p.core_rp_slot = []  # (lo, hi) f32 [128, slots]
    for c in range(n_cores):
        idx_q = [np.zeros((sl * 128,), dtype=np.int16) for sl in p.stream_len]
        dst_rel = np.full((128, n_chunks), -1.0, dtype=np.float32)
        rp_lo = np.zeros((128, slots), dtype=np.float32)
        rp_hi = np.ones((128, slots), dtype=np.float32)
        # consume per-block per-q edge lists chunk by chunk
        consumed = {}
        for ci in range(n_chunks):
            s, q, pos = int(chunk_slot[ci]), int(chunk_q[ci]), int(chunk_pos[ci])
            b = core_blocks[c][s]
            if b < 0:
                continue
            key = (s, q)
            off = consumed.get(key, 0)
            sl_arr, dl_arr = blk_edges[b][q]
            take = sl_arr[off:off + 128]
            if len(take):
                lanes = len(take)
                idx_q[q][pos * 128: pos * 128 + lanes] = take.astype(np.int16)
                dst_rel[:lanes, ci] = dl_arr[off:off + lanes]
            consumed[key] = off + 128
        for s in range(slots):
            b = core_blocks[c][s]
            if b >= 0:
                lo_v = rowptr[b * 128: b * 128 + 128].astype(np.float32)
                hi_v = rowptr[b * 128 + 1: b * 128 + 129].astype(np.float32)
                rp_lo[:, s] = lo_v
                rp_hi[:, s] = np.maximum(hi_v, lo_v + 1)  # ghost nodes: deg>=1
        # wrap idx into [128, len/16] layout (16-wrapped, replicated x8)
        idx_wrapped = []
        for q in range(n_subt):
            if p.stream_len[q] == 0:
                idx_wrapped.append(np.zeros((128, 1), dtype=np.int16))
                continue
            a = idx_q[q].reshape(-1, 16).T  # [16, n/16]
            idx_wrapped.append(np.tile(a, (8, 1)).copy())
        p.core_idx.append(idx_wrapped)
        p.core_dst_rel.append(dst_rel.astype(ml_dtypes.bfloat16))
        p.core_rp_slot.append((rp_lo, rp_hi))

    # global rp arrays for T scaling (f-major [128, n_blocks])
    deg = (rowptr[1:] - rowptr[:-1]).astype(np.float32)
    deg[n_nodes:] = 1.0
    deg[deg == 0] = 1.0  # safety: isolated nodes would have self-loop anyway
    p.rp_deg_g = deg.reshape(n_blocks, 128).T.copy()  # [128, n_blocks] f-major
    p.plan_time = time.time() - t0
    return p


# ---------------------------------------------------------------------------
# Device kernel
# ---------------------------------------------------------------------------


def build_nc(p, n_cores=None, xb=4096, tb=8, ob=8):
    """Build the uniform SPMD Bacc program for plan `p`.
    xb: xT columns per load; tb: T tiles per write; ob: out blocks per write."""
    d = p.d
    n_tiles = p.n_pad // 128
    tb = min(tb, p.align_rows // 128)
    ob = min(ob, p.slots)

    nc = bacc.Bacc("TRN2", target_bir_lowering=False, debug=False,
                   num_devices=n_cores or p.n_cores)

    xT = nc.dram_tensor("xT", [d, p.n_pad], BF16, kind="ExternalInput")
    WT = nc.dram_tensor("WT", [d, d], BF16, kind="ExternalInput")
    bvec = nc.dram_tensor("bvec", [1, d], BF16, kind="ExternalInput")
    iota = nc.dram_tensor("iota", [128, 128], BF16, kind="ExternalInput")
    deg_g = nc.dram_tensor("deg_g", [128, n_tiles], F32, kind="ExternalInput")
    rp_lo = nc.dram_tensor("rp_lo", [128, p.slots], F32, kind="ExternalInput")
    rp_hi = nc.dram_tensor("rp_hi", [128, p.slots], F32, kind="ExternalInput")
    dst_rel = nc.dram_tensor("dst_rel", [128, p.n_chunks], BF16,
                             kind="ExternalInput")
    idx_t = []
    for q in range(p.n_subt):
        cols = max(p.stream_len[q] * 8, 1)  # 128 idx/chunk / 16
        idx_t.append(nc.dram_tensor(f"idx{q}", [128, cols], I16,
                                    kind="ExternalInput"))
    T_t = [nc.dram_tensor(f"T{q}", [p.subt_sizes[q], d], BF16, kind="Internal")
           for q in range(p.n_subt)]
    out_t = nc.dram_tensor("out", [p.slots * 128, d], F32, kind="ExternalOutput")

    with tile.TileContext(nc) as tc, ExitStack() as ctx:
        # ---- P0: constants + degree prep ----
        cpool = ctx.enter_context(tc.tile_pool(name="consts", bufs=1))
        WT_sb = cpool.tile([d, d], BF16)
        nc.sync.dma_start(WT_sb[:], WT.ap()[:, :])
        b_sb = cpool.tile([1, d], BF16)
        nc.sync.dma_start(b_sb[:], bvec.ap()[:, :])
        ones_sb = cpool.tile([1, 128], BF16)
        nc.vector.memset(ones_sb[:], 1.0)
        iota_sb = cpool.tile([128, 128], BF16)
        nc.sync.dma_start(iota_sb[:], iota.ap()[:, :])
        dummy_g = cpool.tile([128, 128], BF16)
        nc.vector.memset(dummy_g[:], 0.0)

        deg_sb = cpool.tile([128, n_tiles], F32)
        nc.sync.dma_start(deg_sb[:], deg_g.ap()[:, :])
        tmp_g = cpool.tile([128, n_tiles], F32)
        nc.scalar.activation(tmp_g[:], deg_sb[:],
                             mybir.ActivationFunctionType.Sqrt)
        rdeg_g = cpool.tile([128, n_tiles], F32)
        nc.vector.reciprocal(rdeg_g[:], tmp_g[:])

        rpl_sb = cpool.tile([128, p.slots], F32)
        nc.sync.dma_start(rpl_sb[:], rp_lo.ap()[:, :])
        rph_sb = cpool.tile([128, p.slots], F32)
        nc.sync.dma_start(rph_sb[:], rp_hi.ap()[:, :])
        degs_sb = cpool.tile([128, p.slots], F32)
        nc.vector.tensor_tensor(out=degs_sb[:], in0=rph_sb[:], in1=rpl_sb[:],
                                op=mybir.AluOpType.subtract)
        tmps_sb = cpool.tile([128, p.slots], F32)
        nc.scalar.activation(tmps_sb[:], degs_sb[:],
                             mybir.ActivationFunctionType.Sqrt)
        rdeg_s = cpool.tile([128, p.slots], F32)
        nc.vector.reciprocal(rdeg_s[:], tmps_sb[:])

        dstrel_sb = cpool.tile([128, p.n_chunks], BF16)
        nc.sync.dma_start(dstrel_sb[:], dst_rel.ap()[:, :])
        idx_sb = []
        for q in range(p.n_subt):
            t = cpool.tile([128, idx_t[q].shape[1]], I16, name=f"idxsb{q}")
            nc.sync.dma_start(t[:], idx_t[q].ap()[:, :])
            idx_sb.append(t)

        # ---- P1: build T = rdeg * (x @ W^T + b) ----
        xpool = ctx.enter_context(tc.tile_pool(name="xT", bufs=2))
        p1psum = ctx.enter_context(tc.tile_pool(name="p1psum", bufs=3,
                                                space="PSUM"))
        tstage = ctx.enter_context(tc.tile_pool(name="tstage", bufs=2))

        n_xb = math.ceil(p.n_pad / xb)
        tiles_per_xb = xb // 128
        stage = None
        for t in range(n_tiles):
            if t % tiles_per_xb == 0:
                x_sb = xpool.tile([d, xb], BF16, name="x_sb")
                c0 = t * 128
                nc.sync.dma_start(x_sb[:, 0:min(xb, p.n_pad - c0)],
                                  xT.ap()[:, c0:min(c0 + xb, p.n_pad)])
            if t % tb == 0:
                stage = tstage.tile([128, tb * d], BF16, name="stage")
            ps = p1psum.tile([128, d], F32, name="p1ps", space="PSUM")
            xoff = (t % tiles_per_xb) * 128
            nc.tensor.matmul(out=ps[:], lhsT=x_sb[:, xoff:xoff + 128],
                             rhs=WT_sb[:], start=True, stop=False)
            nc.tensor.matmul(out=ps[:], lhsT=ones_sb[:1, :], rhs=b_sb[:1, :],
                             start=False, stop=True)
            col = (t % tb) * d
            if t % 2 == 0:
                nc.vector.tensor_scalar_mul(stage[:, col:col + d], ps[:],
                                            rdeg_g[:, t:t + 1])
            else:
                nc.scalar.activation(stage[:, col:col + d], ps[:],
                                     mybir.ActivationFunctionType.Copy,
                                     scale=rdeg_g[:, t:t + 1])
            if t % tb == tb - 1 or t == n_tiles - 1:
                t0b = t // tb * tb
                ntw = t - t0b + 1
                # subtable boundaries are tb-aligned (sizes multiple of tb*128)
                q = int(np.searchsorted(p.subt_off[1:], t0b * 128, side="right"))
                r0 = t0b * 128 - int(p.subt_off[q])
                assert r0 + ntw * 128 <= p.subt_sizes[q], "T write crosses subtable"
                nc.sync.dma_start(
                    out=T_t[q].ap()[r0:r0 + ntw * 128, :].rearrange(
                        "(j p) d -> p j d", p=128),
                    in_=stage[:, 0:ntw * d].rearrange(
                        "p (j d) -> p j d", d=d))

        # ---- P2: gather + selection matmul + scaled drain ----
        gpools = [ctx.enter_context(
            tc.tile_pool(name=f"g{q}", bufs=2)) for q in range(p.n_subt)]
        stpool = ctx.enter_context(tc.tile_pool(name="st", bufs=3))
        p2psum = ctx.enter_context(tc.tile_pool(name="p2psum", bufs=4,
                                                space="PSUM"))
        opool = ctx.enter_context(tc.tile_pool(name="ostage", bufs=2))

        gtiles = {}

        def get_gtile(q, i):
            if (q, i) not in gtiles:
                gt = gpools[q].tile([128, p.gb * d], BF16, name=f"gt{q}")
                nc.gpsimd.dma_gather(
                    out_ap=gt[:].rearrange("p (j d) -> p j d", d=d),
                    in_ap=T_t[q].ap()[:, :],
                    idxs_ap=idx_sb[q][:, i * p.gb * 8:(i + 1) * p.gb * 8],
                    num_idxs=p.gb * 128,
                    num_idxs_reg=p.gb * 128,
                    elem_size=d,
                    single_packet=False,
                )
                gtiles[(q, i)] = gt
            return gtiles[(q, i)]

        st_tile = None
        ostage = None
        ci = 0
        for s in range(p.slots):
            ps2 = p2psum.tile([128, d], F32, name="p2ps", space="PSUM")
            ks = int(p.k_s[s])
            for j in range(ks):
                if ci % p.sg == 0:
                    ng = min(p.sg, p.n_chunks - ci)
                    st_tile = stpool.tile([128, p.sg * 128], BF16, name="st_t")
                    nc.vector.tensor_tensor(
                        out=st_tile[:, 0:ng * 128].rearrange(
                            "p (g i) -> p g i", i=128),
                        in0=dstrel_sb[:, ci:ci + ng].unsqueeze(2).broadcast_to(
                            (128, ng, 128)),
                        in1=iota_sb[:].unsqueeze(1).broadcast_to((128, ng, 128)),
                        op=mybir.AluOpType.is_equal)
                q = int(p.chunk_q[ci])
                pos = int(p.chunk_pos[ci])
                if p.chunk_pos[ci] < 0:
                    rhs = dummy_g[:]
                else:
                    gt = get_gtile(q, pos // p.gb)
                    o = (pos % p.gb) * d
                    rhs = gt[:, o:o + d]
                stoff = (ci % p.sg) * 128
                nc.tensor.matmul(out=ps2[:], lhsT=st_tile[:, stoff:stoff + 128],
                                 rhs=rhs, start=(j == 0), stop=(j == ks - 1))
                ci += 1
            if s % ob == 0:
                ostage = opool.tile([128, ob * d], F32, name="ostage")
            ocol = (s % ob) * d
            nc.scalar.activation(ostage[:, ocol:ocol + d], ps2[:],
                                 mybir.ActivationFunctionType.Copy,
                                 scale=rdeg_s[:, s:s + 1])
            if s % ob == ob - 1 or s == p.slots - 1:
                s0 = s // ob * ob
                nsw = s - s0 + 1
                nc.sync.dma_start(
                    out=out_t.ap()[s0 * 128:(s0 + nsw) * 128, :].rearrange(
                        "(j p) d -> p j d", p=128),
                    in_=ostage[:, 0:nsw * d].rearrange(
                        "p (j d) -> p j d", d=d))
        assert ci == p.n_chunks

    nc.compile()
    return nc


# ---------------------------------------------------------------------------
# Orchestration
# ---------------------------------------------------------------------------


def make_inputs(p, x, W, b):
    d = p.d
    xT = np.zeros((d, p.n_pad), dtype=ml_dtypes.bfloat16)
    xT[:, :p.n_nodes] = np.ascontiguousarray(x.T).astype(ml_dtypes.bfloat16)
    WT = np.ascontiguousarray(W.T).astype(ml_dtypes.bfloat16)
    bvec = np.asarray(b, dtype=np.float32).reshape(1, d).astype(ml_dtypes.bfloat16)
    iota = np.broadcast_to(np.arange(128, dtype=np.float32), (128, 128)).astype(
        ml_dtypes.bfloat16).copy()
    common = {"xT": xT, "WT": WT, "bvec": bvec, "iota": iota,
              "deg_g": p.rp_deg_g}
    in_maps = []
    for c in range(p.n_cores):
        m = dict(common)
        m["rp_lo"] = p.core_rp_slot[c][0]
        m["rp_hi"] = p.core_rp_slot[c][1]
        m["dst_rel"] = p.core_dst_rel[c]
        for q in range(p.n_subt):
            m[f"idx{q}"] = p.core_idx[c][q]
        in_maps.append(m)
    return in_maps


def assemble_output(p, results):
    out = np.zeros((p.n_nodes, p.d), dtype=np.float32)
    for c in range(p.n_cores):
        oc = results[c]["out"]
        for s, b in enumerate(p.core_blocks[c]):
            if b < 0 or b * 128 >= p.n_nodes:
                continue
            lo = b * 128
            hi = min(lo + 128, p.n_nodes)
            out[lo:hi] = oc[s * 128: s * 128 + (hi - lo)]
    return out


def gcn_forward(x, edge_index, W, b, n_cores=8, trace=False, **plan_kw):
    n = x.shape[0]
    src = np.asarray(edge_index[0])
    dst = np.asarray(edge_index[1])
    loop = np.arange(n, dtype=src.dtype)
    src_all = np.concatenate([src, loop])
    dst_all = np.concatenate([dst, loop])
    p = build_plan(src_all, dst_all, n, n_cores, d=W.shape[0], **plan_kw)
    nc = build_nc(p)
    in_maps = make_inputs(p, x, W, b)
    res = run_bass_kernel_spmd(nc, in_maps, core_ids=list(range(n_cores)),
                               trace=trace)
    out = assemble_output(p, [r for r in res.results])
    return out, p, res


# ---------------------------------------------------------------------------
# Harness entry point: full inputs in, full output out.
# ---------------------------------------------------------------------------

N_NODES = 100000
N_EDGES = 1600000
IN_CH = 128
OUT_CH = 128
N_CORES = 8


def kernel(x, edge_index, W, b):
    """GCN layer forward on 8 trn2 NeuronCores. Inputs as in setup_inputs()."""
    x = np.asarray(x, dtype=np.float32)
    edge_index = np.asarray(edge_index)
    W = np.asarray(W, dtype=np.float32)
    b = np.asarray(b, dtype=np.float32)
    out, _p, _res = gcn_forward(x, edge_index, W, b, n_cores=N_CORES)
    return out.astype(np.float32)



# revision 2
# speedup vs baseline: 1.8331x; 1.8331x over previous
"""GCN layer (out = D^-1/2 (A+I) D^-1/2 (x W^T + b)) on 8 trn2 NeuronCores.

Strategy (v2, single device phase):
  Linearity refactor:
    out[dst] = rdeg_dst * ( (sum_{e->dst} rdeg_src * x_src) @ W^T
                            + (sum_{e->dst} rdeg_src) * b )
  - Host: append self-loops, sort edges by dst, partition dst-blocks (128
    nodes) across cores (LPT-balanced), build a core-invariant static
    schedule (uniform SPMD program) with per-core data tables. Supply x
    pre-scaled by rdeg as bf16 subtables (int16-indexable), plus the
    host-computed per-dst rdeg and rdeg-weighted-degree (s2) tables.
  - Device, per dst-block: batched dma_gather of scaled-x rows per edge
    chunk (128 edges), selection matrix via is_equal (dst_rel vs iota),
    PE matmul gt^T @ S accumulating agg^T = [in_ch, dst] into PSUM, then
    agg^T -> bf16, one W matmul + rank-1 s2*b bias matmul per block, and
    a scaled (rdeg_dst) drain to the output rows.
"""

import math
import time
from contextlib import ExitStack

import ml_dtypes
import numpy as np

import concourse.bass as bass
import concourse.tile as tile
from concourse import bacc, mybir
from concourse.bass_utils import run_bass_kernel_spmd

F32 = mybir.dt.float32
BF16 = mybir.dt.bfloat16
I16 = mybir.dt.int16

# ---------------------------------------------------------------------------
# Host-side planning
# ---------------------------------------------------------------------------


class Plan:
    pass


def build_plan(src_all, dst_all, n_nodes, n_cores, d=128,
               subt_cap=32768, gb=32, sg=8):
    """src_all/dst_all: edge endpoints INCLUDING self loops.
    Returns Plan with static schedule + per-core data arrays."""
    t0 = time.time()
    p = Plan()
    p.d = d
    p.gb = gb          # chunks per gather batch
    p.sg = sg          # chunks per is_equal batch
    p.n_nodes = n_nodes
    p.n_cores = n_cores

    n_blocks_real = math.ceil(n_nodes / 128)
    slots = math.ceil(n_blocks_real / n_cores)
    p.slots = slots
    n_blocks = slots * n_cores
    p.n_pad = n_blocks * 128

    # subtable boundaries (each multiple of 128 rows, capacity <= subt_cap)
    n_subt = math.ceil(p.n_pad / subt_cap)
    base = (p.n_pad // n_subt) // 128 * 128
    sizes = [base] * n_subt
    sizes[-1] = p.n_pad - base * (n_subt - 1)
    assert all(0 < s <= subt_cap for s in sizes), sizes
    p.subt_sizes = sizes
    p.subt_off = np.cumsum([0] + sizes)  # [n_subt+1]
    p.n_subt = n_subt

    # sort edges by dst
    order = np.argsort(dst_all, kind="stable")
    dst_s = np.asarray(dst_all)[order].astype(np.int64)
    src_s = np.asarray(src_all)[order].astype(np.int64)
    rowptr = np.searchsorted(dst_s, np.arange(p.n_pad + 1))

    # degrees (self-loops included in the edge stream); ghosts get deg 1
    deg = (rowptr[1:] - rowptr[:-1]).astype(np.float64)
    deg[n_nodes:] = 1.0
    deg[deg == 0] = 1.0
    rdeg = (1.0 / np.sqrt(deg)).astype(np.float32)  # [n_pad]
    p.rdeg = rdeg

    # rdeg-weighted in-degree per dst (bias coefficient)
    s2_g = np.zeros(p.n_pad, dtype=np.float32)
    np.add.at(s2_g, dst_s, rdeg[src_s])

    blk_ptr = rowptr[::128]  # [n_blocks+1]

    # per (block, subtable) edge counts -> chunks
    subt_of_src = np.searchsorted(p.subt_off[1:], src_s, side="right")
    blk_chunks = np.zeros((n_blocks, n_subt), dtype=np.int64)
    blk_edges = []  # per block: list per subt of (src_rel, dst_local) arrays
    for b in range(n_blocks):
        lo, hi = blk_ptr[b], blk_ptr[b + 1]
        qs = subt_of_src[lo:hi]
        per_q = []
        for q in range(n_subt):
            m = qs == q
            sl = src_s[lo:hi][m] - p.subt_off[q]
            dl = dst_s[lo:hi][m] - b * 128
            per_q.append((sl, dl))
            blk_chunks[b, q] = math.ceil(len(sl) / 128) if len(sl) else 0
        blk_edges.append(per_q)

    # LPT assignment of blocks to cores by total chunk count
    tot = blk_chunks.sum(axis=1)
    order_blocks = np.argsort(-tot, kind="stable")
    core_loads = [0] * n_cores
    core_blocks = [[] for _ in range(n_cores)]
    for b in order_blocks:
        c = int(np.argmin(core_loads))
        core_loads[c] += int(tot[b])
        core_blocks[c].append(int(b))
    for c in range(n_cores):
        core_blocks[c].sort(key=lambda b: -int(tot[b]))
        while len(core_blocks[c]) < slots:
            core_blocks[c].append(-1)  # ghost slot
    p.core_blocks = core_blocks  # [n_cores][slots] block id or -1

    # static schedule: k[s][q] = max over cores of chunks for that slot/subt
    k_sq = np.zeros((slots, n_subt), dtype=np.int64)
    for c in range(n_cores):
        for s in range(slots):
            b = core_blocks[c][s]
            if b >= 0:
                k_sq[s] = np.maximum(k_sq[s], blk_chunks[b])
    for s in range(slots):
        if k_sq[s].sum() == 0:
            k_sq[s, 0] = 1  # ensure psum gets start/stop
    p.k_sq = k_sq
    p.k_s = k_sq.sum(axis=1)  # chunks per slot
    n_chunks = int(p.k_s.sum())

    # chunk order (slot-major, subt groups in order) + per-subt stream pos
    chunk_q = np.zeros(n_chunks, dtype=np.int64)
    chunk_slot = np.zeros(n_chunks, dtype=np.int64)
    chunk_pos = np.zeros(n_chunks, dtype=np.int64)
    qcount = [0] * n_subt
    ci = 0
    for s in range(slots):
        for q in range(n_subt):
            for _ in range(int(k_sq[s, q])):
                chunk_q[ci] = q
                chunk_slot[ci] = s
                chunk_pos[ci] = qcount[q]
                qcount[q] += 1
                ci += 1
    assert ci == n_chunks
    p.n_chunks = n_chunks
    p.chunk_q = chunk_q
    p.chunk_slot = chunk_slot
    p.chunk_pos = chunk_pos
    p.stream_len = [math.ceil(qcount[q] / gb) * gb if qcount[q] else 0
                    for q in range(n_subt)]   # in chunks, gb-aligned
    p.n_batches = [sl // gb for sl in p.stream_len]

    # per-core data tables
    p.core_idx = []      # per core: list per q of int16 [128, stream_len*8]
    p.core_dst_rel = []  # per core: bf16 [128, n_chunks]
    p.core_rdeg_s = []   # f32 [128, slots]
    p.core_s2 = []       # bf16 [1, slots*128]
    for c in range(n_cores):
        idx_q = [np.zeros((sl * 128,), dtype=np.int16) for sl in p.stream_len]
        dst_rel = np.full((128, n_chunks), -1.0, dtype=np.float32)
        rdeg_s = np.ones((128, slots), dtype=np.float32)
        s2 = np.zeros((1, slots * 128), dtype=np.float32)
        consumed = {}
        for ci in range(n_chunks):
            s, q, pos = int(chunk_slot[ci]), int(chunk_q[ci]), int(chunk_pos[ci])
            b = core_blocks[c][s]
            if b < 0:
                continue
            key = (s, q)
            off = consumed.get(key, 0)
            sl_arr, dl_arr = blk_edges[b][q]
            take = sl_arr[off:off + 128]
            if len(take):
                lanes = len(take)
                idx_q[q][pos * 128: pos * 128 + lanes] = take.astype(np.int16)
                dst_rel[:lanes, ci] = dl_arr[off:off + lanes]
            consumed[key] = off + 128
        for s in range(slots):
            b = core_blocks[c][s]
            if b >= 0:
                rdeg_s[:, s] = rdeg[b * 128: b * 128 + 128]
                s2[0, s * 128:(s + 1) * 128] = s2_g[b * 128: b * 128 + 128]
        # wrap idx into [128, len/16] layout (16-wrapped, replicated x8)
        idx_wrapped = []
        for q in range(n_subt):
            if p.stream_len[q] == 0:
                idx_wrapped.append(np.zeros((128, 1), dtype=np.int16))
                continue
            a = idx_q[q].reshape(-1, 16).T  # [16, n/16]
            idx_wrapped.append(np.tile(a, (8, 1)).copy())
        p.core_idx.append(idx_wrapped)
        p.core_dst_rel.append(dst_rel.astype(ml_dtypes.bfloat16))
        p.core_rdeg_s.append(rdeg_s)
        p.core_s2.append(s2.astype(ml_dtypes.bfloat16))

    p.plan_time = time.time() - t0
    return p


# ---------------------------------------------------------------------------
# Device kernel
# ---------------------------------------------------------------------------


def build_nc(p, n_cores=None, ob=8):
    """Build the uniform SPMD Bacc program for plan `p`.
    ob: output blocks per write."""
    d = p.d
    ob = min(ob, p.slots)

    nc = bacc.Bacc("TRN2", target_bir_lowering=False, debug=False,
                   num_devices=n_cores or p.n_cores)

    WT = nc.dram_tensor("WT", [d, d], BF16, kind="ExternalInput")
    bvec = nc.dram_tensor("bvec", [1, d], BF16, kind="ExternalInput")
    iota = nc.dram_tensor("iota", [128, 128], BF16, kind="ExternalInput")
    rdeg_s = nc.dram_tensor("rdeg_s", [128, p.slots], F32,
                            kind="ExternalInput")
    s2 = nc.dram_tensor("s2", [1, p.slots * 128], BF16, kind="ExternalInput")
    dst_rel = nc.dram_tensor("dst_rel", [128, p.n_chunks], BF16,
                             kind="ExternalInput")
    idx_t = []
    xq_t = []
    for q in range(p.n_subt):
        cols = max(p.stream_len[q] * 8, 1)  # 128 idx/chunk / 16
        idx_t.append(nc.dram_tensor(f"idx{q}", [128, cols], I16,
                                    kind="ExternalInput"))
        xq_t.append(nc.dram_tensor(f"xq{q}", [p.subt_sizes[q], d], BF16,
                                   kind="ExternalInput"))
    out_t = nc.dram_tensor("out", [p.slots * 128, d], F32,
                           kind="ExternalOutput")

    with tile.TileContext(nc) as tc, ExitStack() as ctx:
        # ---- constants ----
        cpool = ctx.enter_context(tc.tile_pool(name="consts", bufs=1))
        WT_sb = cpool.tile([d, d], BF16)
        nc.sync.dma_start(WT_sb[:], WT.ap()[:, :])
        b_sb = cpool.tile([1, d], BF16)
        nc.sync.dma_start(b_sb[:], bvec.ap()[:, :])
        iota_sb = cpool.tile([128, 128], BF16)
        nc.sync.dma_start(iota_sb[:], iota.ap()[:, :])
        rdeg_sb = cpool.tile([128, p.slots], F32)
        nc.sync.dma_start(rdeg_sb[:], rdeg_s.ap()[:, :])
        s2_sb = cpool.tile([1, p.slots * 128], BF16)
        nc.sync.dma_start(s2_sb[:], s2.ap()[:, :])
        dstrel_sb = cpool.tile([128, p.n_chunks], BF16)
        nc.sync.dma_start(dstrel_sb[:], dst_rel.ap()[:, :])
        idx_sb = []
        for q in range(p.n_subt):
            t = cpool.tile([128, idx_t[q].shape[1]], I16, name=f"idxsb{q}")
            nc.sync.dma_start(t[:], idx_t[q].ap()[:, :])
            idx_sb.append(t)

        # ---- gather + aggregate + per-block W matmul ----
        gpools = [ctx.enter_context(
            tc.tile_pool(name=f"g{q}", bufs=2)) for q in range(p.n_subt)]
        stpool = ctx.enter_context(tc.tile_pool(name="st", bufs=3))
        aggpool = ctx.enter_context(tc.tile_pool(name="agg", bufs=3))
        psumA = ctx.enter_context(tc.tile_pool(name="psumA", bufs=4,
                                               space="PSUM"))
        psumB = ctx.enter_context(tc.tile_pool(name="psumB", bufs=2,
                                               space="PSUM"))
        opool = ctx.enter_context(tc.tile_pool(name="ostage", bufs=2))

        gtiles = {}

        def get_gtile(q, i):
            if (q, i) not in gtiles:
                gt = gpools[q].tile([128, p.gb * d], BF16, name=f"gt{q}")
                nc.gpsimd.dma_gather(
                    out_ap=gt[:].rearrange("p (j d) -> p j d", d=d),
                    in_ap=xq_t[q].ap()[:, :],
                    idxs_ap=idx_sb[q][:, i * p.gb * 8:(i + 1) * p.gb * 8],
                    num_idxs=p.gb * 128,
                    num_idxs_reg=p.gb * 128,
                    elem_size=d,
                    single_packet=False,
                )
                gtiles[(q, i)] = gt
            return gtiles[(q, i)]

        st_tile = None
        ostage = None
        ci = 0
        for s in range(p.slots):
            ps_agg = psumA.tile([128, d], F32, name="ps_agg", space="PSUM")
            ks = int(p.k_s[s])
            for j in range(ks):
                if ci % p.sg == 0:
                    ng = min(p.sg, p.n_chunks - ci)
                    st_tile = stpool.tile([128, p.sg * 128], BF16, name="st_t")
                    nc.vector.tensor_tensor(
                        out=st_tile[:, 0:ng * 128].rearrange(
                            "p (g i) -> p g i", i=128),
                        in0=dstrel_sb[:, ci:ci + ng].unsqueeze(2).broadcast_to(
                            (128, ng, 128)),
                        in1=iota_sb[:].unsqueeze(1).broadcast_to((128, ng, 128)),
                        op=mybir.AluOpType.is_equal)
                q = int(p.chunk_q[ci])
                pos = int(p.chunk_pos[ci])
                gt = get_gtile(q, pos // p.gb)
                o = (pos % p.gb) * d
                stoff = (ci % p.sg) * 128
                # agg^T[in_ch, dst] += gt[e, in_ch]^T @ st[e, dst]
                nc.tensor.matmul(out=ps_agg[:], lhsT=gt[:, o:o + d],
                                 rhs=st_tile[:, stoff:stoff + 128],
                                 start=(j == 0), stop=(j == ks - 1))
                ci += 1
            aggT = aggpool.tile([128, d], BF16, name="aggT")
            nc.scalar.activation(aggT[:], ps_agg[:],
                                 mybir.ActivationFunctionType.Copy)
            out_ps = psumB.tile([128, d], F32, name="out_ps", space="PSUM")
            # out[dst, oc] = agg^T[ic, dst]^T @ W^T[ic, oc] + s2[dst] * b[oc]
            nc.tensor.matmul(out=out_ps[:], lhsT=aggT[:], rhs=WT_sb[:],
                             start=True, stop=False)
            nc.tensor.matmul(out=out_ps[:],
                             lhsT=s2_sb[:1, s * 128:(s + 1) * 128],
                             rhs=b_sb[:1, :], start=False, stop=True)
            if s % ob == 0:
                ostage = opool.tile([128, ob * d], F32, name="ostage")
            ocol = (s % ob) * d
            nc.scalar.activation(ostage[:, ocol:ocol + d], out_ps[:],
                                 mybir.ActivationFunctionType.Copy,
                                 scale=rdeg_sb[:, s:s + 1])
            if s % ob == ob - 1 or s == p.slots - 1:
                s0 = s // ob * ob
                nsw = s - s0 + 1
                nc.sync.dma_start(
                    out=out_t.ap()[s0 * 128:(s0 + nsw) * 128, :].rearrange(
                        "(j p) d -> p j d", p=128),
                    in_=ostage[:, 0:nsw * d].rearrange(
                        "p (j d) -> p j d", d=d))
        assert ci == p.n_chunks

    nc.compile()
    return nc


# ---------------------------------------------------------------------------
# Orchestration
# ---------------------------------------------------------------------------


def make_inputs(p, x, W, b):
    d = p.d
    xs = np.zeros((p.n_pad, d), dtype=np.float32)
    xs[:p.n_nodes] = np.asarray(x, dtype=np.float32)
    xs *= p.rdeg[:, None]
    xs_bf = xs.astype(ml_dtypes.bfloat16)
    WT = np.ascontiguousarray(np.asarray(W, dtype=np.float32).T).astype(
        ml_dtypes.bfloat16)
    bvec = np.asarray(b, dtype=np.float32).reshape(1, d).astype(
        ml_dtypes.bfloat16)
    iota = np.broadcast_to(np.arange(128, dtype=np.float32),
                           (128, 128)).astype(ml_dtypes.bfloat16).copy()
    common = {"WT": WT, "bvec": bvec, "iota": iota}
    for q in range(p.n_subt):
        lo, hi = int(p.subt_off[q]), int(p.subt_off[q + 1])
        common[f"xq{q}"] = np.ascontiguousarray(xs_bf[lo:hi])
    in_maps = []
    for c in range(p.n_cores):
        m = dict(common)
        m["rdeg_s"] = p.core_rdeg_s[c]
        m["s2"] = p.core_s2[c]
        m["dst_rel"] = p.core_dst_rel[c]
        for q in range(p.n_subt):
            m[f"idx{q}"] = p.core_idx[c][q]
        in_maps.append(m)
    return in_maps


def assemble_output(p, results):
    out = np.zeros((p.n_nodes, p.d), dtype=np.float32)
    for c in range(p.n_cores):
        oc = results[c]["out"]
        for s, b in enumerate(p.core_blocks[c]):
            if b < 0 or b * 128 >= p.n_nodes:
                continue
            lo = b * 128
            hi = min(lo + 128, p.n_nodes)
            out[lo:hi] = oc[s * 128: s * 128 + (hi - lo)]
    return out


def gcn_forward(x, edge_index, W, b, n_cores=8, trace=False, **plan_kw):
    n = x.shape[0]
    src = np.asarray(edge_index[0])
    dst = np.asarray(edge_index[1])
    loop = np.arange(n, dtype=src.dtype)
    src_all = np.concatenate([src, loop])
    dst_all = np.concatenate([dst, loop])
    p = build_plan(src_all, dst_all, n, n_cores, d=W.shape[0], **plan_kw)
    nc = build_nc(p)
    in_maps = make_inputs(p, x, W, b)
    res = run_bass_kernel_spmd(nc, in_maps, core_ids=list(range(n_cores)),
                               trace=trace)
    out = assemble_output(p, [r for r in res.results])
    return out, p, res


# ---------------------------------------------------------------------------
# Harness entry point: full inputs in, full output out.
# ---------------------------------------------------------------------------

N_NODES = 100000
N_EDGES = 1600000
IN_CH = 128
OUT_CH = 128
N_CORES = 8


def kernel(x, edge_index, W, b):
    """GCN layer forward on 8 trn2 NeuronCores. Inputs as in setup_inputs()."""
    x = np.asarray(x, dtype=np.float32)
    edge_index = np.asarray(edge_index)
    W = np.asarray(W, dtype=np.float32)
    b = np.asarray(b, dtype=np.float32)
    out, _p, _res = gcn_forward(x, edge_index, W, b, n_cores=N_CORES)
    return out.astype(np.float32)
